# revision 77
# baseline (speedup 1.0000x reference)
"""Trainium2 Bass kernel for nn_Attention_86698209837214.

Multi-head attention: out = softmax(q k^T / 8) v @ W_out + b_out with
B=4, N=2048, DIM=1024, H=16, Dh=64.

Sharding: 8 cores = (batch b in 0..3) x (head-half hh in 0..1); each core
computes 8 heads of one batch. Host pre-transposes x[b], slices weights and
converts everything to bf16; host adds the two head-half partial outputs
plus b_out.

Device dataflow per core (bf16 operands, fp32 PSUM):
  1. v = x @ Wv in natural [n, c] layout, augmented with a ones column per
     head slot (row 64 of the attn@v accumulator = softmax denominator).
     Second pass is k-outer with inline staging; kT ct0's quarters ride
     between its groups to fill input-DMA bubbles.
  2. qT, kT = (x @ Wq/Wk)^T in [c, n] layout (lhsT = W tiles).
  3. Attention, ic (i-chunk of 512) outer, hp (head pair) inner:
     dots^T per head via K=64 row-packed matmuls (tile_position r0=64*s, the
     two head streams run on disjoint PE row groups and overlap),
     exp split across two engines: ScalarE table exp (bf16 out, s=0 + jp0's
     s=1) and DVE Schraudolph exp (tensor_scalar mult+add -> round-to-int16
     == bf16 bit pattern; ~2% weight noise, acceptable), attn@v accumulated
     over j in PSUM with M=65 (65th row = denominator); av lags dots by
     AV_LAG steps. The dots PSUM ring is 3 tiles deep (6 banks) + av 2
     banks = all 8: each exp gets a ~1.5-step deadline before its tile is
     overwritten, which is what keeps the PE at ~2.4GHz.
     Epilogue: av -> one merged SBUF tile (DVE s0 / ScalarE s1), den row
     hopped out as [8,128] (sync DMA), fast reciprocal on DVE, recip row
     bounced SBUF->DRAM->SBUF where the return DMA has a stride-0 outer dim
     = a free 64-way partition broadcast (GpSimd cannot touch PSUM and its
     partition_broadcast lives in a different Q7 lib than tensor ops — lib
     swaps cost ~6us), normalize muls on GpSimd (DMA-free so its
     TENSOR_TENSOR lib stays resident), s=1 half reaches aT partitions
     64:127 via SBUF-to-SBUF DMA (sync).
  4. Out-projection runs as its own dense PE-bound phase after attention
     (16 n-tiles x 4 head-pair K-accum in the freed dots ring; nt order
     puts ic=3 last so the final epilogue chain hides under it); bf16
     staging copies alternate ScalarE/DVE, results DMA'd out on sync.

Engines: PE near-saturated; ScalarE/DVE are ~90-95% busy with exp (the
hard wall: dots f32 PSUM can only be read by these two engines at ~1
elem/ns/partition); GpSimd takes the normalize; sync+scalar HWDGE queues
carry all DMA. fp8 q/k (DoubleRow) was tried and reverted: softmax-weight
noise lands 1:1 on the output (~5e-2 rel err).
"""

import sys

for _p in ("/opt/trn_rl_repo",):
    if _p not in sys.path:
        sys.path.append(_p)

from contextlib import ExitStack

import ml_dtypes
import numpy as np

import concourse.bass as bass  # noqa: F401
import concourse.tile as tile
from concourse import bacc, mybir
from concourse.bass_utils import run_bass_kernel_spmd

F32 = mybir.dt.float32
F32R = mybir.dt.float32r
BF16 = mybir.dt.bfloat16
F8 = mybir.dt.float8e4
I16 = mybir.dt.int16
AF = mybir.ActivationFunctionType
ALU = mybir.AluOpType
DR = mybir.MatmulPerfMode.DoubleRow

P = 128
NSEQ = 2048  # sequence length per batch
D = 1024  # model dim
CH = 512  # per-core head-dim width (8 heads x 64)
DH = 64
NPAIR = 4  # head pairs per core (c-tiles of 128)
NDT = D // P  # 8 d-tiles
NNT = NSEQ // P  # 16 n-tiles
NNC = NSEQ // 512  # 4 i-chunks
NJP = NNT // 2  # 8 jp steps per chunk
SCALE = 0.125  # DIM_HEAD ** -0.5

# NOTE: fp8 q/k projections were tried (DoubleRow, 4x throughput, ~15us
# saved) but cost ~5e-2 rel err: per-weight exponent noise from quantized
# q,k transfers 1:1 to the output (attn@v is itself a random walk over j,
# so softmax-weight noise does NOT average out). Keep projections bf16.
EXP_SCALE = SCALE

# Schraudolph exp in bf16-bit-pattern domain (scale folded in):
#   int16 bits = round(dots * EXP_SCALE * 2^7/ln2 + (127*128 - 486411/2^16))
A_SCH = EXP_SCALE * 184.6650390625
B_SCH = 16248.576

# (jp, s) pairs whose exp runs on DVE (Schraudolph); rest on ScalarE.
# 9/7 split: ScalarE's table exp is ~25% cheaper per tile than DVE's
# Schraudolph, and DVE also carries half the chunk epilogue. Scalar's one
# double-exp step sits at jp=0, inside the epilogue window.
DVE_EXP = {(jp, 1) for jp in range(1, NSEQ // 256)}


def build_program():
    nc = bacc.Bacc("TRN2", target_bir_lowering=False, debug=False)

    # host-shuffled layouts: partition-major so each partition's DMA run is
    # long and contiguous (DMA engines are packet-rate-bound on short runs)
    xt = nc.dram_tensor("xt", [P, NDT, NSEQ], BF16, kind="ExternalInput")
    wqkv = nc.dram_tensor("wqkv", [P, 3, NDT, CH], BF16, kind="ExternalInput")
    wout = nc.dram_tensor("wout", [P, NPAIR, D], BF16, kind="ExternalInput")
    ones_in = nc.dram_tensor("ones", [P, 1], BF16, kind="ExternalInput")
    out = nc.dram_tensor("out", [NSEQ, D], BF16, kind="ExternalOutput")
    # DRAM bounce buffer for the recip-row partition broadcast: DMA the row
    # out, DMA it back with a stride-0 outer dim (legal for DRAM-side APs).
    # Rotated 3-deep so consecutive chunks never WAR each other.
    rscratch = nc.dram_tensor("rscratch", [3, 1024], F32, kind="Internal")

    out_t = out.ap().rearrange("(nt p) e -> nt p e", p=P)  # [16, 128, 1024]

    copy_flip = [0]

    with tile.TileContext(nc) as tc, ExitStack() as ctx:
        # ---- persistent pools ----
        p_qk = ctx.enter_context(tc.tile_pool(name="p_qk", bufs=1))  # 32 KB/p
        p_v = ctx.enter_context(tc.tile_pool(name="p_v", bufs=1))  # ~16 KB/p
        p_small = ctx.enter_context(tc.tile_pool(name="p_small", bufs=1))
        # PSUM: dots 3x[128,1024] (6 banks) + av 2x[65,512] (2 banks).
        # 3-deep dots buffering gives each exp a ~1.5-step deadline instead of
        # gating the very next step's dots — the out-projection runs as its
        # own phase after attention so it needs no concurrent PSUM.
        ps_mm = ctx.enter_context(tc.tile_pool(name="ps_mm", bufs=3, space="PSUM"))
        ps_av = ctx.enter_context(tc.tile_pool(name="ps_av", bufs=2, space="PSUM"))
        # attention-phase pools (created before the transient phase-A pools so
        # pool release order stays LIFO)
        p_exp = ctx.enter_context(tc.tile_pool(name="p_exp", bufs=12))  # 24 KB/p
        p_aT = ctx.enter_context(tc.tile_pool(name="p_aT", bufs=16))  # 16 KB/p
        p_wout = ctx.enter_context(tc.tile_pool(name="p_wout", bufs=1))  # 8 KB/p
        p_avsb = ctx.enter_context(tc.tile_pool(name="p_avsb", bufs=3))  # 6 KB/p
        p_recip = ctx.enter_context(tc.tile_pool(name="p_recip", bufs=2))
        p_bcast = ctx.enter_context(tc.tile_pool(name="p_bcast", bufs=3))
        p_tmp = ctx.enter_context(tc.tile_pool(name="p_tmp", bufs=2))
        p_ostage = ctx.enter_context(tc.tile_pool(name="p_ostage", bufs=3))

        ones = p_small.tile([P, 1], BF16, tag="ones")
        nc.sync.dma_start(out=ones, in_=ones_in.ap())
        # dummy exp: pulls the ~2.7us ACT_TABLE_LOAD into the initial DMA wait
        warm = p_small.tile([P, 1], F32, tag="warm")
        nc.scalar.activation(out=warm, in_=ones, func=AF.Exp, scale=1.0)
        # warm the gpsimd TENSOR_TENSOR library while inputs stream in, so the
        # one-time Q7 lib load never lands in the attention epilogue path
        gp_warm = p_small.tile([1, 1], F32, tag="gp_warm")
        nc.gpsimd.tensor_mul(gp_warm, warm[0:1, :], warm[0:1, :])
        def stage_copy(dst, src):
            # alternate PSUM->SBUF staging copies between DVE and ScalarE
            copy_flip[0] ^= 1
            if copy_flip[0]:
                nc.vector.tensor_copy(dst, src)
            else:
                nc.scalar.copy(dst, src)

        # ---- phase A: load xt, wv, wk; compute v_aug ----
        st_xt = ExitStack()
        p_xt = st_xt.enter_context(tc.tile_pool(name="p_xt", bufs=1))  # 32 KB/p
        st_wk = ExitStack()
        p_wk = st_wk.enter_context(tc.tile_pool(name="p_wk", bufs=1))
        p_wq = st_wk.enter_context(tc.tile_pool(name="p_wq", bufs=1))
        st_wv = ExitStack()
        p_wv = st_wv.enter_context(tc.tile_pool(name="p_wv", bufs=1))

        # input DMAs: per-dt slices of xt and wv round-robin across the two
        # HWDGE queues, so the first tiles land ~6us in and the dt-outer
        # v-proj below starts streaming long before the full load completes.
        # (gpsimd issues NO DMAs at all — SWDGE would evict its tensor-op lib
        # from Q7 IRAM.)
        wv_sb = p_wv.tile([P, NDT, CH], BF16, tag="wv")
        xt_sb = p_xt.tile([P, NDT, NSEQ], BF16, tag="xt")
        wk_sb = p_wk.tile([P, NDT, CH], BF16, tag="wk")
        wq_sb = p_wq.tile([P, NDT, CH], BF16, tag="wq")
        dma_q = [nc.scalar, nc.sync]
        # column-quartered AND half-pass-major: all first-half quarters for
        # every dt land before any second-half quarter, so the v-proj's first
        # pass (which needs q0/q1 of ALL dts) never queues behind data the
        # second pass won't touch until ~10us later
        for dt_i in range(NDT):
            q = dma_q[dt_i % 2]
            q.dma_start(out=wv_sb[:, dt_i], in_=wqkv.ap()[:, 2, dt_i])
            for cq in range(2):
                q.dma_start(
                    out=xt_sb[:, dt_i, cq * 512 : (cq + 1) * 512],
                    in_=xt.ap()[:, dt_i, cq * 512 : (cq + 1) * 512],
                )
        for dt_i in range(NDT):
            q = dma_q[dt_i % 2]
            for cq in range(2, 4):
                q.dma_start(
                    out=xt_sb[:, dt_i, cq * 512 : (cq + 1) * 512],
                    in_=xt.ap()[:, dt_i, cq * 512 : (cq + 1) * 512],
                )
        # wk/wq halves ride both queues after the xt streams: landed by ~20us
        # on every core (kT starts ~28us) without delaying the v-proj's data
        nc.scalar.dma_start(out=wk_sb[:, 0:4], in_=wqkv.ap()[:, 1, 0:4])
        nc.sync.dma_start(out=wk_sb[:, 4:8], in_=wqkv.ap()[:, 1, 4:8])
        nc.scalar.dma_start(out=wq_sb[:, 0:4], in_=wqkv.ap()[:, 0, 0:4])
        nc.sync.dma_start(out=wq_sb[:, 4:8], in_=wqkv.ap()[:, 0, 4:8])
        xt_tiles = [xt_sb[:, dt_i] for dt_i in range(NDT)]
        wv_tiles = [wv_sb[:, dt_i] for dt_i in range(NDT)]
        wk_tiles = [wk_sb[:, dt_i] for dt_i in range(NDT)]
        wq_tiles = [wq_sb[:, dt_i] for dt_i in range(NDT)]

        # v_aug: per head-slot sg, 65 cols = [v_sg (64) | ones (1)]
        v_tiles = []
        for nt in range(NNT):
            dst = p_v.tile([P, 8 * 65], BF16, tag=f"v{nt}")
            ones_dst = dst.rearrange("p (h c) -> p h c", c=65)[:, :, 64:65]
            nc.vector.memset(ones_dst, 1.0)
            v_tiles.append(dst)
        # dt-outer over 8 parallel PSUM accumulators (all 8 banks are free at
        # startup): each dt's matmuls issue as soon as that dt slice arrives
        vacc = []
        for nm in ("vA", "vB", "vD"):
            t = ps_mm.tile([P, 1024], F32, tag="mm", name=nm)
            vacc += [t[:, 0:512], t[:, 512:1024]]
        for s in range(2):
            t = ps_av.tile([P, 512], F32, tag="av", name=f"vC{s}")
            vacc.append(t)
        def stage_v(half, k):
            dst = v_tiles[half * 8 + k]
            v_dst = dst.rearrange("p (h c) -> p h c", c=65)[:, :, 0:DH]
            stage_copy(v_dst, vacc[k].rearrange("p (h c) -> p h c", c=DH))

        # first pass dt-outer: each dt's matmuls issue as soon as that dt
        # slice lands, streaming behind the input DMA
        for dt_i in range(NDT):
            for k in range(8):
                nc.tensor.matmul(
                    vacc[k],
                    xt_tiles[dt_i][:, k * P : (k + 1) * P],
                    wv_tiles[dt_i],
                    start=(dt_i == 0),
                    stop=(dt_i == NDT - 1),
                )
        for k in range(8):
            stage_v(0, k)
        # ---- phase B emitters (kT/qT c-tiles) ----
        kT_tiles = []
        qT_tiles = []

        def emit_qk_quarter(which, ct, nch, dst):
            w_tiles = wq_tiles if which == "q" else wk_tiles
            woff = ct * P
            acc = ps_mm.tile([P, 512], F32, tag="mm", name="acc")
            for dt_i in range(NDT):
                nc.tensor.matmul(
                    acc,
                    w_tiles[dt_i][:, woff : woff + P],
                    xt_tiles[dt_i][:, nch * 512 : (nch + 1) * 512],
                    start=(dt_i == 0),
                    stop=(dt_i == NDT - 1),
                )
            stage_copy(dst[:, nch * 512 : (nch + 1) * 512], acc)

        def alloc_qk(which, ct):
            dst = p_qk.tile([P, NSEQ], BF16, tag=f"{which}T{ct}", name=f"{which}T{ct}")
            (kT_tiles if which == "k" else qT_tiles).append(dst)
            return dst

        def emit_qk_tile(which, ct):
            dst = alloc_qk(which, ct)
            for nch in range(NNC):
                emit_qk_quarter(which, ct, nch, dst)

        # second pass k-outer: data is resident by now, so finish group k and
        # stage it while group k+1 still matmuls — the kT phase's PSUM ring
        # slots free up 6 groups earlier (was a ~3.6us PE gap + HAM
        # re-throttle when all 8 copies piled up at the end). kT ct0's four
        # quarters ride between the last v groups: when a v group waits on
        # late xt quarters, the PE chews on kT instead of idling.
        kT0 = alloc_qk("k", 0)
        for k in range(8):
            nt = 8 + k
            for dt_i in range(NDT):
                nc.tensor.matmul(
                    vacc[k],
                    xt_tiles[dt_i][:, nt * P : (nt + 1) * P],
                    wv_tiles[dt_i],
                    start=(dt_i == 0),
                    stop=(dt_i == NDT - 1),
                )
            stage_v(1, k)
            if 3 <= k <= 6:
                emit_qk_quarter("k", 0, k - 3, kT0)
        st_wv.close()

        # ---- phase B: remaining kT c-tiles, then all qT tiles ----
        # qT ct1-3 used to interleave between the first attention chunks;
        # that was free when attention was engine-bound, but with the PE now
        # the binding resource their staging copies only disturbed the exp
        # deadlines mid-flight. Emit them in the dense pre-phase instead.
        for ct in range(1, NPAIR):
            emit_qk_tile("k", ct)
        for ct in range(NPAIR):
            emit_qk_tile("q", ct)

        wout_sb = p_wout.tile([P, NPAIR, D], BF16, tag="wout")
        nc.sync.dma_start(out=wout_sb, in_=wout.ap())
        wout_tiles = [wout_sb[:, ct] for ct in range(NPAIR)]

        # ---- attention: flat software pipeline across chunk boundaries ----
        # Per jp step of the CURRENT chunk: dots -> av of the step one behind
        # (possibly the previous chunk's tail) -> maybe one out-proj group ->
        # exp for this step. The PE's in-order queue therefore never waits on
        # an epilogue chain: epilogues and out-projections overlap the next
        # chunk's dots/av stream (>3.4us PE-idle gaps re-throttle HAM to
        # 1.2GHz, which is what made the naive ordering slow).
        aT_by_ic = {}  # ic -> [aT tile per hp]

        class Cctx:
            def __init__(self, ic, hp):
                self.ic, self.hp, self.i0 = ic, hp, ic * 512
                self.ci = ic * NPAIR + hp
                self.av_ps = [
                    ps_av.tile([65, 512], F32, tag="av", name=f"av{s}")
                    for s in range(2)
                ]
                self.aT = p_aT.tile([P, 512], BF16, tag="aT", name=f"aT{ic}_{hp}")
                aT_by_ic.setdefault(ic, []).append(self.aT)

        def emit_dots(c, jp):
            tiles = []
            for s in range(2):
                tiles.append(ps_mm.tile([P, 1024], F32, tag="mm", name="dots"))
            for half in range(2):
                for s in range(2):
                    r0 = s * DH
                    jtx = 2 * jp + half
                    nc.tensor.matmul(
                        tiles[s][:, half * 512 : (half + 1) * 512],
                        kT_tiles[c.hp][r0 : r0 + DH, jtx * P : (jtx + 1) * P],
                        qT_tiles[c.hp][r0 : r0 + DH, c.i0 : c.i0 + 512],
                        start=True,
                        stop=True,
                        tile_position=(r0, 0),
                    )
            return tiles

        def emit_exp(c, jp, dots_tiles):
            exp_tiles = []
            for s in range(2):
                e = p_exp.tile([P, 1024], BF16, tag="exp")
                if (jp, s) in DVE_EXP:
                    nc.vector.tensor_scalar(
                        out=e.bitcast(I16),
                        in0=dots_tiles[s],
                        scalar1=A_SCH,
                        scalar2=B_SCH,
                        op0=ALU.mult,
                        op1=ALU.add,
                    )
                else:
                    nc.scalar.activation(
                        out=e, in_=dots_tiles[s], func=AF.Exp, scale=EXP_SCALE
                    )
                exp_tiles.append(e)
            return exp_tiles

        def emit_av(c, jp, exp_pair):
            for s in range(2):
                sg = c.hp * 2 + s
                for half in range(2):
                    jtx = 2 * jp + half
                    nc.tensor.matmul(
                        c.av_ps[s],
                        v_tiles[jtx][:, sg * 65 : sg * 65 + 65],
                        exp_pair[s][:, half * 512 : (half + 1) * 512],
                        start=(jp == 0 and half == 0),
                        stop=(jp == NJP - 1 and half == 1),
                    )

        # Epilogue pipeline. GpSimd is DMA-free after the startup loads and
        # cannot touch PSUM, so its ext-isa ucode stays resident and it does
        # the SBUF-only normalize divides; ScalarE evacuates av PSUM; the
        # denominator row is replicated across partitions by a sync-queue DMA
        # reading the av_sb row with a stride-0 partition broadcast. No
        # reciprocal, no casting DMA, no PE broadcast matmul.
        def emit_epilogue_a(c, e):
            # evacuate av PSUM into one merged SBUF tile (one copy per
            # engine, in parallel); hop the denominator row out as [8,128]
            # so the reciprocal runs on eight partitions at 1/8 the cost.
            # (Quarter-width copies split across both engines were tried to
            # cut the bank-release latency — 80us SLOWER, don't.)
            av_sb = p_avsb.tile([65, 1024], F32, tag="av_sb", name="avsb")
            nc.vector.tensor_copy(av_sb[:, 0:512], c.av_ps[0])
            nc.scalar.copy(av_sb[:, 512:1024], c.av_ps[1])
            e["av_sb"] = av_sb
            den_sb = p_recip.tile([8, 128], F32, tag="den_sb")
            nc.sync.dma_start(out=den_sb, in_=av_sb[64:65, :])
            e["den_sb"] = den_sb

        def emit_epilogue_b1(c, e):
            # fast reciprocal on DVE, then bounce the rows to DRAM (sync);
            # DRAM layout comes out linear: recip_s0[512] | recip_s1[512]
            recip = p_recip.tile([8, 128], F32, tag="recip")
            nc.vector.reciprocal_approx_fast(out=recip, in_=e["den_sb"])
            e["scratch"] = rscratch.ap()[c.ci % 3 : c.ci % 3 + 1]
            nc.sync.dma_start(out=e["scratch"], in_=recip)

        def emit_epilogue_b2(c, e):
            # DMA the recip row back with a stride-0 outer dim: a 64-way
            # partition replicate with no engine work at all
            bc = p_bcast.tile([DH, 1024], F32, tag="bc", name="bc")
            nc.sync.dma_start(out=bc, in_=e["scratch"].partition_broadcast(DH))
            e["bc"] = bc

        def emit_epilogue_b3(c, e):
            # normalize on GpSimd (bf16 out); s=1 reaches aT partitions
            # 64:127 via an SBUF-to-SBUF DMA hop (engines can't cross
            # partitions)
            av_sb, bc = e["av_sb"], e["bc"]
            nc.gpsimd.tensor_mul(c.aT[0:DH, :], av_sb[0:DH, 0:512], bc[:, 0:512])
            tmp = p_tmp.tile([DH, 512], BF16, tag="tmp")
            nc.gpsimd.tensor_mul(tmp, av_sb[0:DH, 512:1024], bc[:, 512:1024])
            nc.sync.dma_start(out=c.aT[DH:P, :], in_=tmp)

        from collections import deque

        pend_av = deque()  # (cctx, jp, exp_tiles); av lags dots by AV_LAG steps
        AV_LAG = 4  # exp gets four pipeline steps of slack before av needs it
        pend_tasks = []  # [countdown, fn] epilogue stages

        def pump(drain=False):
            if pend_av and (len(pend_av) >= AV_LAG or drain):
                pc, pjp, pexp = pend_av.popleft()
                emit_av(pc, pjp, pexp)
                if pjp == NJP - 1:
                    e = {}
                    emit_epilogue_a(pc, e)
                    for delay, fn in (
                        (2, emit_epilogue_b1),
                        (4, emit_epilogue_b2),
                        (6, emit_epilogue_b3),
                    ):
                        pend_tasks.append([delay, fn, pc, e])
            for t in list(pend_tasks):
                t[0] -= 1
                if t[0] <= 0:
                    t[1](t[2], t[3])
                    pend_tasks.remove(t)

        chunks = [(ic, hp) for ic in range(NNC) for hp in range(NPAIR)]
        for ci, (ic, hp) in enumerate(chunks):
            c = Cctx(ic, hp)
            for jp in range(NJP):
                dots_tiles = emit_dots(c, jp)
                pump()
                exp_tiles = emit_exp(c, jp, dots_tiles)
                pend_av.append((c, jp, exp_tiles))
        # drain av tail + epilogues
        while pend_av or pend_tasks:
            pump(drain=True)

        # ---- out-projection phase: all 16 n-tiles after attention ----
        # Runs as a dense PE-bound burst (64 matmuls of N=1024, ~27us).
        # Keeping it out of the attention loop frees 2 PSUM banks, which the
        # dots pool uses for 3-deep buffering — that breaks the
        # dots->exp->dots PSUM-reuse serialization that dominated the
        # attention steady state. nt order puts ic=3 last, so the final
        # chunk's epilogue chain hides behind ~20us of earlier out-proj work.
        for nt in range(NNT):
            ic, lnt = nt // 4, nt % 4
            o_ps = ps_mm.tile([P, D], F32, tag="mm", name=f"o{nt}")
            for ec in range(2):
                for hp in range(NPAIR):
                    nc.tensor.matmul(
                        o_ps[:, ec * 512 : (ec + 1) * 512],
                        aT_by_ic[ic][hp][:, lnt * P : (lnt + 1) * P],
                        wout_tiles[hp][:, ec * 512 : (ec + 1) * 512],
                        start=(hp == 0),
                        stop=(hp == NPAIR - 1),
                    )
            o_sb = p_ostage.tile([P, D], BF16, tag="o_sb", name=f"o_sb{nt}")
            stage_copy(o_sb, o_ps)
            nc.sync.dma_start(out=out_t[nt], in_=o_sb)

        st_wk.close()
        st_xt.close()

    nc.compile()
    return nc


_NC = None


def _get_program():
    global _NC
    if _NC is None:
        _NC = build_program()
    return _NC


INNER = 1024
BF = ml_dtypes.bfloat16


def kernel(x, W_qkv, W_out, b_out):
    x = np.asarray(x, dtype=np.float32)
    W_qkv = np.asarray(W_qkv, dtype=np.float32)
    W_out = np.asarray(W_out, dtype=np.float32)
    b_out = np.asarray(b_out, dtype=np.float32)
    B = x.shape[0]

    nc = _get_program()
    in_maps = []
    for b in range(B):
        # partition-major shuffles so per-partition DMA runs are contiguous
        xt_sh = np.ascontiguousarray(
            x[b].T.reshape(NDT, P, NSEQ).transpose(1, 0, 2)
        ).astype(BF)
        for hh in range(2):
            cs = hh * CH
            wq = W_qkv[:, cs : cs + CH]
            wk = W_qkv[:, INNER + cs : INNER + cs + CH]
            wv = W_qkv[:, 2 * INNER + cs : 2 * INNER + cs + CH]
            wqkv_sh = np.ascontiguousarray(
                np.stack([wq, wk, wv])  # [3, 1024, 512]
                .reshape(3, NDT, P, CH)
                .transpose(2, 0, 1, 3)
            ).astype(BF)
            wout_sh = np.ascontiguousarray(
                W_out[cs : cs + CH, :].reshape(NPAIR, P, D).transpose(1, 0, 2)
            ).astype(BF)
            in_maps.append(
                {
                    "xt": xt_sh,
                    "wqkv": wqkv_sh,
                    "wout": wout_sh,
                    "ones": np.ones((P, 1), dtype=BF),
                }
            )
    res = run_bass_kernel_spmd(nc, in_maps, core_ids=list(range(8)))
    out = np.empty((B, NSEQ, D), dtype=np.float32)
    for b in range(B):
        out[b] = (
            res.results[2 * b]["out"].astype(np.float32)
            + res.results[2 * b + 1]["out"].astype(np.float32)
            + b_out
        )
    return out



# revision 78
# speedup vs baseline: 1.0040x; 1.0040x over previous
"""Trainium2 Bass kernel for nn_Attention_86698209837214.

Multi-head attention: out = softmax(q k^T / 8) v @ W_out + b_out with
B=4, N=2048, DIM=1024, H=16, Dh=64.

Sharding: 8 cores = (batch b in 0..3) x (head-half hh in 0..1); each core
computes 8 heads of one batch. Host pre-transposes x[b], slices weights and
converts everything to bf16; host adds the two head-half partial outputs
plus b_out.

Device dataflow per core (bf16 operands, fp32 PSUM):
  1. v = x @ Wv in natural [n, c] layout, augmented with a ones column per
     head slot (row 64 of the attn@v accumulator = softmax denominator).
     Second pass is k-outer with inline staging; kT ct0's quarters ride
     between its groups to fill input-DMA bubbles.
  2. qT, kT = (x @ Wq/Wk)^T in [c, n] layout (lhsT = W tiles).
  3. Attention, ic (i-chunk of 512) outer, hp (head pair) inner:
     dots^T per head via K=64 row-packed matmuls (tile_position r0=64*s, the
     two head streams run on disjoint PE row groups and overlap),
     exp split across two engines: ScalarE table exp (bf16 out, s=0 + jp0's
     s=1) and DVE Schraudolph exp (tensor_scalar mult+add -> round-to-int16
     == bf16 bit pattern; ~2% weight noise, acceptable), attn@v accumulated
     over j in PSUM with M=65 (65th row = denominator); av lags dots by
     AV_LAG steps. The dots PSUM ring is 3 tiles deep (6 banks) + av 2
     banks = all 8: each exp gets a ~1.5-step deadline before its tile is
     overwritten, which is what keeps the PE at ~2.4GHz.
     Epilogue: av -> one merged SBUF tile (DVE s0 / ScalarE s1), den row
     hopped out as [8,128] (sync DMA), fast reciprocal on DVE, recip row
     bounced SBUF->DRAM->SBUF where the return DMA has a stride-0 outer dim
     = a free 64-way partition broadcast (GpSimd cannot touch PSUM and its
     partition_broadcast lives in a different Q7 lib than tensor ops — lib
     swaps cost ~6us), normalize muls on GpSimd (DMA-free so its
     TENSOR_TENSOR lib stays resident), s=1 half reaches aT partitions
     64:127 via SBUF-to-SBUF DMA (sync).
  4. Out-projection runs as its own dense PE-bound phase after attention
     (16 n-tiles x 4 head-pair K-accum in the freed dots ring; nt order
     puts ic=3 last so the final epilogue chain hides under it); bf16
     staging copies alternate ScalarE/DVE, results DMA'd out on sync.

Engines: PE near-saturated; ScalarE/DVE are ~90-95% busy with exp (the
hard wall: dots f32 PSUM can only be read by these two engines at ~1
elem/ns/partition); GpSimd takes the normalize; sync+scalar HWDGE queues
carry all DMA. fp8 q/k (DoubleRow) was tried and reverted: softmax-weight
noise lands 1:1 on the output (~5e-2 rel err).
"""

import sys

for _p in ("/opt/trn_rl_repo",):
    if _p not in sys.path:
        sys.path.append(_p)

from contextlib import ExitStack

import ml_dtypes
import numpy as np

import concourse.bass as bass  # noqa: F401
import concourse.tile as tile
from concourse import bacc, mybir
from concourse.bass_utils import run_bass_kernel_spmd

F32 = mybir.dt.float32
F32R = mybir.dt.float32r
BF16 = mybir.dt.bfloat16
F8 = mybir.dt.float8e4
I16 = mybir.dt.int16
AF = mybir.ActivationFunctionType
ALU = mybir.AluOpType
DR = mybir.MatmulPerfMode.DoubleRow

P = 128
NSEQ = 2048  # sequence length per batch
D = 1024  # model dim
CH = 512  # per-core head-dim width (8 heads x 64)
DH = 64
NPAIR = 4  # head pairs per core (c-tiles of 128)
NDT = D // P  # 8 d-tiles
NNT = NSEQ // P  # 16 n-tiles
NNC = NSEQ // 512  # 4 i-chunks
NJP = NNT // 2  # 8 jp steps per chunk
SCALE = 0.125  # DIM_HEAD ** -0.5

# NOTE: fp8 q/k projections were tried (DoubleRow, 4x throughput, ~15us
# saved) but cost ~5e-2 rel err: per-weight exponent noise from quantized
# q,k transfers 1:1 to the output (attn@v is itself a random walk over j,
# so softmax-weight noise does NOT average out). Keep projections bf16.
EXP_SCALE = SCALE

# Schraudolph exp in bf16-bit-pattern domain (scale folded in):
#   int16 bits = round(dots * EXP_SCALE * 2^7/ln2 + (127*128 - 486411/2^16))
A_SCH = EXP_SCALE * 184.6650390625
B_SCH = 16248.576

# (jp, s) pairs whose exp runs on DVE (Schraudolph); rest on ScalarE.
# 9/7 split: ScalarE's table exp is ~25% cheaper per tile than DVE's
# Schraudolph, and DVE also carries half the chunk epilogue. Scalar's one
# double-exp step sits at jp=0, inside the epilogue window.
DVE_EXP = {(jp, 1) for jp in range(1, NSEQ // 256)}


def build_program():
    nc = bacc.Bacc("TRN2", target_bir_lowering=False, debug=False)

    # host-shuffled layouts: partition-major so each partition's DMA run is
    # long and contiguous (DMA engines are packet-rate-bound on short runs)
    xt = nc.dram_tensor("xt", [P, NDT, NSEQ], BF16, kind="ExternalInput")
    wqkv = nc.dram_tensor("wqkv", [P, 3, NDT, CH], BF16, kind="ExternalInput")
    wout = nc.dram_tensor("wout", [P, NPAIR, D], BF16, kind="ExternalInput")
    ones_in = nc.dram_tensor("ones", [P, 1], BF16, kind="ExternalInput")
    out = nc.dram_tensor("out", [NSEQ, D], BF16, kind="ExternalOutput")
    # DRAM bounce buffer for the recip-row partition broadcast: DMA the row
    # out, DMA it back with a stride-0 outer dim (legal for DRAM-side APs).
    # Rotated 3-deep so consecutive chunks never WAR each other.
    rscratch = nc.dram_tensor("rscratch", [3, 1024], F32, kind="Internal")

    out_t = out.ap().rearrange("(nt p) e -> nt p e", p=P)  # [16, 128, 1024]

    copy_flip = [0]

    with tile.TileContext(nc) as tc, ExitStack() as ctx:
        # ---- persistent pools ----
        p_qk = ctx.enter_context(tc.tile_pool(name="p_qk", bufs=1))  # 32 KB/p
        p_v = ctx.enter_context(tc.tile_pool(name="p_v", bufs=1))  # ~16 KB/p
        p_small = ctx.enter_context(tc.tile_pool(name="p_small", bufs=1))
        # PSUM: dots 3x[128,1024] (6 banks) + av 2x[65,512] (2 banks).
        # 3-deep dots buffering gives each exp a ~1.5-step deadline instead of
        # gating the very next step's dots — the out-projection runs as its
        # own phase after attention so it needs no concurrent PSUM.
        ps_mm = ctx.enter_context(tc.tile_pool(name="ps_mm", bufs=3, space="PSUM"))
        ps_av = ctx.enter_context(tc.tile_pool(name="ps_av", bufs=2, space="PSUM"))
        # attention-phase pools (created before the transient phase-A pools so
        # pool release order stays LIFO)
        p_exp = ctx.enter_context(tc.tile_pool(name="p_exp", bufs=12))  # 24 KB/p
        p_aT = ctx.enter_context(tc.tile_pool(name="p_aT", bufs=16))  # 16 KB/p
        p_wout = ctx.enter_context(tc.tile_pool(name="p_wout", bufs=1))  # 8 KB/p
        p_avsb = ctx.enter_context(tc.tile_pool(name="p_avsb", bufs=3))  # 6 KB/p
        p_recip = ctx.enter_context(tc.tile_pool(name="p_recip", bufs=2))
        p_bcast = ctx.enter_context(tc.tile_pool(name="p_bcast", bufs=3))
        p_tmp = ctx.enter_context(tc.tile_pool(name="p_tmp", bufs=2))
        p_ostage = ctx.enter_context(tc.tile_pool(name="p_ostage", bufs=3))

        ones = p_small.tile([P, 1], BF16, tag="ones")
        nc.sync.dma_start(out=ones, in_=ones_in.ap())
        # dummy exp: pulls the ~2.7us ACT_TABLE_LOAD into the initial DMA wait
        warm = p_small.tile([P, 1], F32, tag="warm")
        nc.scalar.activation(out=warm, in_=ones, func=AF.Exp, scale=1.0)
        # warm the gpsimd TENSOR_TENSOR library while inputs stream in, so the
        # one-time Q7 lib load never lands in the attention epilogue path
        gp_warm = p_small.tile([1, 1], F32, tag="gp_warm")
        nc.gpsimd.tensor_mul(gp_warm, warm[0:1, :], warm[0:1, :])
        def stage_copy(dst, src):
            # alternate PSUM->SBUF staging copies between DVE and ScalarE
            copy_flip[0] ^= 1
            if copy_flip[0]:
                nc.vector.tensor_copy(dst, src)
            else:
                nc.scalar.copy(dst, src)

        # ---- phase A: load xt, wv, wk; compute v_aug ----
        st_xt = ExitStack()
        p_xt = st_xt.enter_context(tc.tile_pool(name="p_xt", bufs=1))  # 32 KB/p
        st_wk = ExitStack()
        p_wk = st_wk.enter_context(tc.tile_pool(name="p_wk", bufs=1))
        p_wq = st_wk.enter_context(tc.tile_pool(name="p_wq", bufs=1))
        st_wv = ExitStack()
        p_wv = st_wv.enter_context(tc.tile_pool(name="p_wv", bufs=1))

        # input DMAs: per-dt slices of xt and wv round-robin across the two
        # HWDGE queues, so the first tiles land ~6us in and the dt-outer
        # v-proj below starts streaming long before the full load completes.
        # (gpsimd issues NO DMAs at all — SWDGE would evict its tensor-op lib
        # from Q7 IRAM.)
        wv_sb = p_wv.tile([P, NDT, CH], BF16, tag="wv")
        xt_sb = p_xt.tile([P, NDT, NSEQ], BF16, tag="xt")
        wk_sb = p_wk.tile([P, NDT, CH], BF16, tag="wk")
        wq_sb = p_wq.tile([P, NDT, CH], BF16, tag="wq")
        dma_q = [nc.scalar, nc.sync]
        # column-quartered AND half-pass-major: all first-half quarters for
        # every dt land before any second-half quarter, so the v-proj's first
        # pass (which needs q0/q1 of ALL dts) never queues behind data the
        # second pass won't touch until ~10us later
        for dt_i in range(NDT):
            q = dma_q[dt_i % 2]
            q.dma_start(out=wv_sb[:, dt_i], in_=wqkv.ap()[:, 2, dt_i])
            for cq in range(2):
                q.dma_start(
                    out=xt_sb[:, dt_i, cq * 512 : (cq + 1) * 512],
                    in_=xt.ap()[:, dt_i, cq * 512 : (cq + 1) * 512],
                )
        for dt_i in range(NDT):
            q = dma_q[dt_i % 2]
            for cq in range(2, 4):
                q.dma_start(
                    out=xt_sb[:, dt_i, cq * 512 : (cq + 1) * 512],
                    in_=xt.ap()[:, dt_i, cq * 512 : (cq + 1) * 512],
                )
        # wk/wq halves ride both queues after the xt streams: landed by ~20us
        # on every core (kT starts ~28us) without delaying the v-proj's data
        nc.scalar.dma_start(out=wk_sb[:, 0:4], in_=wqkv.ap()[:, 1, 0:4])
        nc.sync.dma_start(out=wk_sb[:, 4:8], in_=wqkv.ap()[:, 1, 4:8])
        nc.scalar.dma_start(out=wq_sb[:, 0:4], in_=wqkv.ap()[:, 0, 0:4])
        nc.sync.dma_start(out=wq_sb[:, 4:8], in_=wqkv.ap()[:, 0, 4:8])
        xt_tiles = [xt_sb[:, dt_i] for dt_i in range(NDT)]
        wv_tiles = [wv_sb[:, dt_i] for dt_i in range(NDT)]
        wk_tiles = [wk_sb[:, dt_i] for dt_i in range(NDT)]
        wq_tiles = [wq_sb[:, dt_i] for dt_i in range(NDT)]

        # v_aug: per head-slot sg, 65 cols = [v_sg (64) | ones (1)]
        v_tiles = []
        for nt in range(NNT):
            dst = p_v.tile([P, 8 * 65], BF16, tag=f"v{nt}")
            ones_dst = dst.rearrange("p (h c) -> p h c", c=65)[:, :, 64:65]
            nc.vector.memset(ones_dst, 1.0)
            v_tiles.append(dst)
        # dt-outer over 8 parallel PSUM accumulators (all 8 banks are free at
        # startup): each dt's matmuls issue as soon as that dt slice arrives
        vacc = []
        for nm in ("vA", "vB", "vD"):
            t = ps_mm.tile([P, 1024], F32, tag="mm", name=nm)
            vacc += [t[:, 0:512], t[:, 512:1024]]
        for s in range(2):
            t = ps_av.tile([P, 512], F32, tag="av", name=f"vC{s}")
            vacc.append(t)
        def stage_v(half, k):
            dst = v_tiles[half * 8 + k]
            v_dst = dst.rearrange("p (h c) -> p h c", c=65)[:, :, 0:DH]
            stage_copy(v_dst, vacc[k].rearrange("p (h c) -> p h c", c=DH))

        # first pass dt-outer: each dt's matmuls issue as soon as that dt
        # slice lands, streaming behind the input DMA
        for dt_i in range(NDT):
            for k in range(8):
                nc.tensor.matmul(
                    vacc[k],
                    xt_tiles[dt_i][:, k * P : (k + 1) * P],
                    wv_tiles[dt_i],
                    start=(dt_i == 0),
                    stop=(dt_i == NDT - 1),
                )
        for k in range(8):
            stage_v(0, k)
        # ---- phase B emitters (kT/qT c-tiles) ----
        kT_tiles = []
        qT_tiles = []

        def emit_qk_quarter(which, ct, nch, dst):
            w_tiles = wq_tiles if which == "q" else wk_tiles
            woff = ct * P
            acc = ps_mm.tile([P, 512], F32, tag="mm", name="acc")
            for dt_i in range(NDT):
                nc.tensor.matmul(
                    acc,
                    w_tiles[dt_i][:, woff : woff + P],
                    xt_tiles[dt_i][:, nch * 512 : (nch + 1) * 512],
                    start=(dt_i == 0),
                    stop=(dt_i == NDT - 1),
                )
            stage_copy(dst[:, nch * 512 : (nch + 1) * 512], acc)

        def alloc_qk(which, ct):
            dst = p_qk.tile([P, NSEQ], BF16, tag=f"{which}T{ct}", name=f"{which}T{ct}")
            (kT_tiles if which == "k" else qT_tiles).append(dst)
            return dst

        def emit_qk_tile(which, ct):
            dst = alloc_qk(which, ct)
            for nch in range(NNC):
                emit_qk_quarter(which, ct, nch, dst)

        # second pass k-outer: data is resident by now, so finish group k and
        # stage it while group k+1 still matmuls — the kT phase's PSUM ring
        # slots free up 6 groups earlier (was a ~3.6us PE gap + HAM
        # re-throttle when all 8 copies piled up at the end). kT ct0's four
        # quarters ride between the last v groups: when a v group waits on
        # late xt quarters, the PE chews on kT instead of idling.
        kT0 = alloc_qk("k", 0)
        for k in range(8):
            nt = 8 + k
            for dt_i in range(NDT):
                nc.tensor.matmul(
                    vacc[k],
                    xt_tiles[dt_i][:, nt * P : (nt + 1) * P],
                    wv_tiles[dt_i],
                    start=(dt_i == 0),
                    stop=(dt_i == NDT - 1),
                )
            stage_v(1, k)
            if 3 <= k <= 6:
                emit_qk_quarter("k", 0, k - 3, kT0)
        st_wv.close()

        # ---- phase B: remaining kT c-tiles, then all qT tiles ----
        # qT ct1-3 used to interleave between the first attention chunks;
        # that was free when attention was engine-bound, but with the PE now
        # the binding resource their staging copies only disturbed the exp
        # deadlines mid-flight. Emit them in the dense pre-phase instead.
        for ct in range(1, NPAIR):
            emit_qk_tile("k", ct)
        for ct in range(NPAIR):
            emit_qk_tile("q", ct)

        wout_sb = p_wout.tile([P, NPAIR, D], BF16, tag="wout")
        nc.sync.dma_start(out=wout_sb, in_=wout.ap())
        wout_tiles = [wout_sb[:, ct] for ct in range(NPAIR)]

        # ---- attention: flat software pipeline across chunk boundaries ----
        # Per jp step of the CURRENT chunk: dots -> av of the step one behind
        # (possibly the previous chunk's tail) -> maybe one out-proj group ->
        # exp for this step. The PE's in-order queue therefore never waits on
        # an epilogue chain: epilogues and out-projections overlap the next
        # chunk's dots/av stream (>3.4us PE-idle gaps re-throttle HAM to
        # 1.2GHz, which is what made the naive ordering slow).
        aT_by_ic = {}  # ic -> [aT tile per hp]

        class Cctx:
            def __init__(self, ic, hp):
                self.ic, self.hp, self.i0 = ic, hp, ic * 512
                self.ci = ic * NPAIR + hp
                self.av_ps = [
                    ps_av.tile([65, 512], F32, tag="av", name=f"av{s}")
                    for s in range(2)
                ]
                self.aT = p_aT.tile([P, 512], BF16, tag="aT", name=f"aT{ic}_{hp}")
                aT_by_ic.setdefault(ic, []).append(self.aT)

        def emit_dots(c, jp):
            tiles = []
            for s in range(2):
                tiles.append(ps_mm.tile([P, 1024], F32, tag="mm", name="dots"))
            for half in range(2):
                for s in range(2):
                    r0 = s * DH
                    jtx = 2 * jp + half
                    nc.tensor.matmul(
                        tiles[s][:, half * 512 : (half + 1) * 512],
                        kT_tiles[c.hp][r0 : r0 + DH, jtx * P : (jtx + 1) * P],
                        qT_tiles[c.hp][r0 : r0 + DH, c.i0 : c.i0 + 512],
                        start=True,
                        stop=True,
                        tile_position=(r0, 0),
                    )
            return tiles

        def emit_exp(c, jp, dots_tiles):
            exp_tiles = []
            for s in range(2):
                e = p_exp.tile([P, 1024], BF16, tag="exp")
                if (jp, s) in DVE_EXP:
                    nc.vector.tensor_scalar(
                        out=e.bitcast(I16),
                        in0=dots_tiles[s],
                        scalar1=A_SCH,
                        scalar2=B_SCH,
                        op0=ALU.mult,
                        op1=ALU.add,
                    )
                else:
                    nc.scalar.activation(
                        out=e, in_=dots_tiles[s], func=AF.Exp, scale=EXP_SCALE
                    )
                exp_tiles.append(e)
            return exp_tiles

        def emit_av(c, jp, exp_pair):
            for s in range(2):
                sg = c.hp * 2 + s
                for half in range(2):
                    jtx = 2 * jp + half
                    nc.tensor.matmul(
                        c.av_ps[s],
                        v_tiles[jtx][:, sg * 65 : sg * 65 + 65],
                        exp_pair[s][:, half * 512 : (half + 1) * 512],
                        start=(jp == 0 and half == 0),
                        stop=(jp == NJP - 1 and half == 1),
                    )

        # Epilogue pipeline. GpSimd is DMA-free after the startup loads and
        # cannot touch PSUM, so its ext-isa ucode stays resident and it does
        # the SBUF-only normalize divides; ScalarE evacuates av PSUM; the
        # denominator row is replicated across partitions by a sync-queue DMA
        # reading the av_sb row with a stride-0 partition broadcast. No
        # reciprocal, no casting DMA, no PE broadcast matmul.
        def emit_epilogue_a(c, e):
            # evacuate av PSUM into one merged SBUF tile (one copy per
            # engine, in parallel); hop the denominator row out as [8,128]
            # so the reciprocal runs on eight partitions at 1/8 the cost.
            # (Quarter-width copies split across both engines were tried to
            # cut the bank-release latency — 80us SLOWER, don't.)
            av_sb = p_avsb.tile([65, 1024], F32, tag="av_sb", name="avsb")
            nc.vector.tensor_copy(av_sb[:, 0:512], c.av_ps[0])
            nc.scalar.copy(av_sb[:, 512:1024], c.av_ps[1])
            e["av_sb"] = av_sb
            den_sb = p_recip.tile([8, 128], F32, tag="den_sb")
            nc.sync.dma_start(out=den_sb, in_=av_sb[64:65, :])
            e["den_sb"] = den_sb

        def emit_epilogue_b1(c, e):
            # fast reciprocal on DVE, then bounce the rows to DRAM (sync);
            # DRAM layout comes out linear: recip_s0[512] | recip_s1[512]
            recip = p_recip.tile([8, 128], F32, tag="recip")
            nc.vector.reciprocal_approx_fast(out=recip, in_=e["den_sb"])
            e["scratch"] = rscratch.ap()[c.ci % 3 : c.ci % 3 + 1]
            nc.sync.dma_start(out=e["scratch"], in_=recip)

        def emit_epilogue_b2(c, e):
            # DMA the recip row back with a stride-0 outer dim: a 64-way
            # partition replicate with no engine work at all
            bc = p_bcast.tile([DH, 1024], F32, tag="bc", name="bc")
            nc.sync.dma_start(out=bc, in_=e["scratch"].partition_broadcast(DH))
            e["bc"] = bc

        def emit_epilogue_b3(c, e):
            # normalize on GpSimd (bf16 out); s=1 reaches aT partitions
            # 64:127 via an SBUF-to-SBUF DMA hop (engines can't cross
            # partitions)
            av_sb, bc = e["av_sb"], e["bc"]
            nc.gpsimd.tensor_mul(c.aT[0:DH, :], av_sb[0:DH, 0:512], bc[:, 0:512])
            tmp = p_tmp.tile([DH, 512], BF16, tag="tmp")
            nc.gpsimd.tensor_mul(tmp, av_sb[0:DH, 512:1024], bc[:, 512:1024])
            nc.sync.dma_start(out=c.aT[DH:P, :], in_=tmp)

        from collections import deque

        pend_av = deque()  # (cctx, jp, exp_tiles); av lags dots by AV_LAG steps
        AV_LAG = 5  # exp gets five pipeline steps of slack before av needs it
        pend_tasks = []  # [countdown, fn] epilogue stages

        def pump(drain=False):
            if pend_av and (len(pend_av) >= AV_LAG or drain):
                pc, pjp, pexp = pend_av.popleft()
                emit_av(pc, pjp, pexp)
                if pjp == NJP - 1:
                    e = {}
                    emit_epilogue_a(pc, e)
                    for delay, fn in (
                        (2, emit_epilogue_b1),
                        (4, emit_epilogue_b2),
                        (6, emit_epilogue_b3),
                    ):
                        pend_tasks.append([delay, fn, pc, e])
            for t in list(pend_tasks):
                t[0] -= 1
                if t[0] <= 0:
                    t[1](t[2], t[3])
                    pend_tasks.remove(t)

        chunks = [(ic, hp) for ic in range(NNC) for hp in range(NPAIR)]
        for ci, (ic, hp) in enumerate(chunks):
            c = Cctx(ic, hp)
            for jp in range(NJP):
                dots_tiles = emit_dots(c, jp)
                pump()
                exp_tiles = emit_exp(c, jp, dots_tiles)
                pend_av.append((c, jp, exp_tiles))
        # drain av tail + epilogues
        while pend_av or pend_tasks:
            pump(drain=True)

        # ---- out-projection phase: all 16 n-tiles after attention ----
        # Runs as a dense PE-bound burst (64 matmuls of N=1024, ~27us).
        # Keeping it out of the attention loop frees 2 PSUM banks, which the
        # dots pool uses for 3-deep buffering — that breaks the
        # dots->exp->dots PSUM-reuse serialization that dominated the
        # attention steady state. nt order puts ic=3 last, so the final
        # chunk's epilogue chain hides behind ~20us of earlier out-proj work.
        for nt in range(NNT):
            ic, lnt = nt // 4, nt % 4
            o_ps = ps_mm.tile([P, D], F32, tag="mm", name=f"o{nt}")
            for ec in range(2):
                for hp in range(NPAIR):
                    nc.tensor.matmul(
                        o_ps[:, ec * 512 : (ec + 1) * 512],
                        aT_by_ic[ic][hp][:, lnt * P : (lnt + 1) * P],
                        wout_tiles[hp][:, ec * 512 : (ec + 1) * 512],
                        start=(hp == 0),
                        stop=(hp == NPAIR - 1),
                    )
            o_sb = p_ostage.tile([P, D], BF16, tag="o_sb", name=f"o_sb{nt}")
            stage_copy(o_sb, o_ps)
            nc.sync.dma_start(out=out_t[nt], in_=o_sb)

        st_wk.close()
        st_xt.close()

    nc.compile()
    return nc


_NC = None


def _get_program():
    global _NC
    if _NC is None:
        _NC = build_program()
    return _NC


INNER = 1024
BF = ml_dtypes.bfloat16


def kernel(x, W_qkv, W_out, b_out):
    x = np.asarray(x, dtype=np.float32)
    W_qkv = np.asarray(W_qkv, dtype=np.float32)
    W_out = np.asarray(W_out, dtype=np.float32)
    b_out = np.asarray(b_out, dtype=np.float32)
    B = x.shape[0]

    nc = _get_program()
    in_maps = []
    for b in range(B):
        # partition-major shuffles so per-partition DMA runs are contiguous
        xt_sh = np.ascontiguousarray(
            x[b].T.reshape(NDT, P, NSEQ).transpose(1, 0, 2)
        ).astype(BF)
        for hh in range(2):
            cs = hh * CH
            wq = W_qkv[:, cs : cs + CH]
            wk = W_qkv[:, INNER + cs : INNER + cs + CH]
            wv = W_qkv[:, 2 * INNER + cs : 2 * INNER + cs + CH]
            wqkv_sh = np.ascontiguousarray(
                np.stack([wq, wk, wv])  # [3, 1024, 512]
                .reshape(3, NDT, P, CH)
                .transpose(2, 0, 1, 3)
            ).astype(BF)
            wout_sh = np.ascontiguousarray(
                W_out[cs : cs + CH, :].reshape(NPAIR, P, D).transpose(1, 0, 2)
            ).astype(BF)
            in_maps.append(
                {
                    "xt": xt_sh,
                    "wqkv": wqkv_sh,
                    "wout": wout_sh,
                    "ones": np.ones((P, 1), dtype=BF),
                }
            )
    res = run_bass_kernel_spmd(nc, in_maps, core_ids=list(range(8)))
    out = np.empty((B, NSEQ, D), dtype=np.float32)
    for b in range(B):
        out[b] = (
            res.results[2 * b]["out"].astype(np.float32)
            + res.results[2 * b + 1]["out"].astype(np.float32)
            + b_out
        )
    return out



# revision 80
# speedup vs baseline: 1.0046x; 1.0006x over previous
"""Trainium2 Bass kernel for nn_Attention_86698209837214.

Multi-head attention: out = softmax(q k^T / 8) v @ W_out + b_out with
B=4, N=2048, DIM=1024, H=16, Dh=64.

Sharding: 8 cores = (batch b in 0..3) x (head-half hh in 0..1); each core
computes 8 heads of one batch. Host pre-transposes x[b], slices weights and
converts everything to bf16; host adds the two head-half partial outputs
plus b_out.

Device dataflow per core (bf16 operands, fp32 PSUM):
  1. v = x @ Wv in natural [n, c] layout, augmented with a ones column per
     head slot (row 64 of the attn@v accumulator = softmax denominator).
     Second pass is k-outer with inline staging; kT ct0's quarters ride
     between its groups to fill input-DMA bubbles.
  2. qT, kT = (x @ Wq/Wk)^T in [c, n] layout (lhsT = W tiles).
  3. Attention, ic (i-chunk of 512) outer, hp (head pair) inner:
     dots^T per head via K=64 row-packed matmuls (tile_position r0=64*s, the
     two head streams run on disjoint PE row groups and overlap),
     exp split across two engines: ScalarE table exp (bf16 out, s=0 + jp0's
     s=1) and DVE Schraudolph exp (tensor_scalar mult+add -> round-to-int16
     == bf16 bit pattern; ~2% weight noise, acceptable), attn@v accumulated
     over j in PSUM with M=65 (65th row = denominator); av lags dots by
     AV_LAG steps. The dots PSUM ring is 3 tiles deep (6 banks) + av 2
     banks = all 8: each exp gets a ~1.5-step deadline before its tile is
     overwritten, which is what keeps the PE at ~2.4GHz.
     Epilogue: av -> one merged SBUF tile (DVE s0 / ScalarE s1), den row
     hopped out as [8,128] (sync DMA), fast reciprocal on DVE, recip row
     bounced SBUF->DRAM->SBUF where the return DMA has a stride-0 outer dim
     = a free 64-way partition broadcast (GpSimd cannot touch PSUM and its
     partition_broadcast lives in a different Q7 lib than tensor ops — lib
     swaps cost ~6us), normalize muls on GpSimd (DMA-free so its
     TENSOR_TENSOR lib stays resident), s=1 half reaches aT partitions
     64:127 via SBUF-to-SBUF DMA (sync).
  4. Out-projection runs as its own dense PE-bound phase after attention
     (16 n-tiles x 4 head-pair K-accum in the freed dots ring; nt order
     puts ic=3 last so the final epilogue chain hides under it); bf16
     staging copies alternate ScalarE/DVE, results DMA'd out on sync.

Engines: PE near-saturated; ScalarE/DVE are ~90-95% busy with exp (the
hard wall: dots f32 PSUM can only be read by these two engines at ~1
elem/ns/partition); GpSimd takes the normalize; sync+scalar HWDGE queues
carry all DMA. fp8 q/k (DoubleRow) was tried and reverted: softmax-weight
noise lands 1:1 on the output (~5e-2 rel err).
"""

import sys

for _p in ("/opt/trn_rl_repo",):
    if _p not in sys.path:
        sys.path.append(_p)

from contextlib import ExitStack

import ml_dtypes
import numpy as np

import concourse.bass as bass  # noqa: F401
import concourse.tile as tile
from concourse import bacc, mybir
from concourse.bass_utils import run_bass_kernel_spmd

F32 = mybir.dt.float32
F32R = mybir.dt.float32r
BF16 = mybir.dt.bfloat16
F8 = mybir.dt.float8e4
I16 = mybir.dt.int16
AF = mybir.ActivationFunctionType
ALU = mybir.AluOpType
DR = mybir.MatmulPerfMode.DoubleRow

P = 128
NSEQ = 2048  # sequence length per batch
D = 1024  # model dim
CH = 512  # per-core head-dim width (8 heads x 64)
DH = 64
NPAIR = 4  # head pairs per core (c-tiles of 128)
NDT = D // P  # 8 d-tiles
NNT = NSEQ // P  # 16 n-tiles
NNC = NSEQ // 512  # 4 i-chunks
NJP = NNT // 2  # 8 jp steps per chunk
SCALE = 0.125  # DIM_HEAD ** -0.5

# NOTE: fp8 q/k projections were tried (DoubleRow, 4x throughput, ~15us
# saved) but cost ~5e-2 rel err: per-weight exponent noise from quantized
# q,k transfers 1:1 to the output (attn@v is itself a random walk over j,
# so softmax-weight noise does NOT average out). Keep projections bf16.
EXP_SCALE = SCALE

# Schraudolph exp in bf16-bit-pattern domain (scale folded in):
#   int16 bits = round(dots * EXP_SCALE * 2^7/ln2 + (127*128 - 486411/2^16))
A_SCH = EXP_SCALE * 184.6650390625
B_SCH = 16248.576

# (jp, s) pairs whose exp runs on DVE (Schraudolph); rest on ScalarE.
# 9/7 split: ScalarE's table exp is ~25% cheaper per tile than DVE's
# Schraudolph, and DVE also carries half the chunk epilogue. Scalar's one
# double-exp step sits at jp=0, inside the epilogue window.
DVE_EXP = {(jp, 1) for jp in range(1, NSEQ // 256)}


def build_program():
    nc = bacc.Bacc("TRN2", target_bir_lowering=False, debug=False)

    # host-shuffled layouts: partition-major so each partition's DMA run is
    # long and contiguous (DMA engines are packet-rate-bound on short runs)
    xt = nc.dram_tensor("xt", [P, NDT, NSEQ], BF16, kind="ExternalInput")
    wqkv = nc.dram_tensor("wqkv", [P, 3, NDT, CH], BF16, kind="ExternalInput")
    wout = nc.dram_tensor("wout", [P, NPAIR, D], BF16, kind="ExternalInput")
    ones_in = nc.dram_tensor("ones", [P, 1], BF16, kind="ExternalInput")
    out = nc.dram_tensor("out", [NSEQ, D], BF16, kind="ExternalOutput")
    # DRAM bounce buffer for the recip-row partition broadcast: DMA the row
    # out, DMA it back with a stride-0 outer dim (legal for DRAM-side APs).
    # Rotated 3-deep so consecutive chunks never WAR each other.
    rscratch = nc.dram_tensor("rscratch", [3, 1024], F32, kind="Internal")

    out_t = out.ap().rearrange("(nt p) e -> nt p e", p=P)  # [16, 128, 1024]

    copy_flip = [0]

    with tile.TileContext(nc) as tc, ExitStack() as ctx:
        # ---- persistent pools ----
        p_qk = ctx.enter_context(tc.tile_pool(name="p_qk", bufs=1))  # 32 KB/p
        p_v = ctx.enter_context(tc.tile_pool(name="p_v", bufs=1))  # ~16 KB/p
        p_small = ctx.enter_context(tc.tile_pool(name="p_small", bufs=1))
        # PSUM: dots 3x[128,1024] (6 banks) + av 2x[65,512] (2 banks).
        # 3-deep dots buffering gives each exp a ~1.5-step deadline instead of
        # gating the very next step's dots — the out-projection runs as its
        # own phase after attention so it needs no concurrent PSUM.
        ps_mm = ctx.enter_context(tc.tile_pool(name="ps_mm", bufs=3, space="PSUM"))
        ps_av = ctx.enter_context(tc.tile_pool(name="ps_av", bufs=2, space="PSUM"))
        # attention-phase pools (created before the transient phase-A pools so
        # pool release order stays LIFO)
        p_exp = ctx.enter_context(tc.tile_pool(name="p_exp", bufs=12))  # 24 KB/p
        p_aT = ctx.enter_context(tc.tile_pool(name="p_aT", bufs=16))  # 16 KB/p
        p_wout = ctx.enter_context(tc.tile_pool(name="p_wout", bufs=1))  # 8 KB/p
        p_avsb = ctx.enter_context(tc.tile_pool(name="p_avsb", bufs=3))  # 6 KB/p
        p_recip = ctx.enter_context(tc.tile_pool(name="p_recip", bufs=2))
        p_bcast = ctx.enter_context(tc.tile_pool(name="p_bcast", bufs=3))
        p_tmp = ctx.enter_context(tc.tile_pool(name="p_tmp", bufs=2))
        p_ostage = ctx.enter_context(tc.tile_pool(name="p_ostage", bufs=3))

        ones = p_small.tile([P, 1], BF16, tag="ones")
        nc.sync.dma_start(out=ones, in_=ones_in.ap())
        # dummy exp: pulls the ~2.7us ACT_TABLE_LOAD into the initial DMA wait
        warm = p_small.tile([P, 1], F32, tag="warm")
        nc.scalar.activation(out=warm, in_=ones, func=AF.Exp, scale=1.0)
        # warm the gpsimd TENSOR_TENSOR library while inputs stream in, so the
        # one-time Q7 lib load never lands in the attention epilogue path
        gp_warm = p_small.tile([1, 1], F32, tag="gp_warm")
        nc.gpsimd.tensor_mul(gp_warm, warm[0:1, :], warm[0:1, :])
        def stage_copy(dst, src):
            # alternate PSUM->SBUF staging copies between DVE and ScalarE
            copy_flip[0] ^= 1
            if copy_flip[0]:
                nc.vector.tensor_copy(dst, src)
            else:
                nc.scalar.copy(dst, src)

        # ---- phase A: load xt, wv, wk; compute v_aug ----
        st_xt = ExitStack()
        p_xt = st_xt.enter_context(tc.tile_pool(name="p_xt", bufs=1))  # 32 KB/p
        st_wk = ExitStack()
        p_wk = st_wk.enter_context(tc.tile_pool(name="p_wk", bufs=1))
        p_wq = st_wk.enter_context(tc.tile_pool(name="p_wq", bufs=1))
        st_wv = ExitStack()
        p_wv = st_wv.enter_context(tc.tile_pool(name="p_wv", bufs=1))

        # input DMAs: per-dt slices of xt and wv round-robin across the two
        # HWDGE queues, so the first tiles land ~6us in and the dt-outer
        # v-proj below starts streaming long before the full load completes.
        # (gpsimd issues NO DMAs at all — SWDGE would evict its tensor-op lib
        # from Q7 IRAM.)
        wv_sb = p_wv.tile([P, NDT, CH], BF16, tag="wv")
        xt_sb = p_xt.tile([P, NDT, NSEQ], BF16, tag="xt")
        wk_sb = p_wk.tile([P, NDT, CH], BF16, tag="wk")
        wq_sb = p_wq.tile([P, NDT, CH], BF16, tag="wq")
        dma_q = [nc.scalar, nc.sync]
        # column-quartered AND half-pass-major: all first-half quarters for
        # every dt land before any second-half quarter, so the v-proj's first
        # pass (which needs q0/q1 of ALL dts) never queues behind data the
        # second pass won't touch until ~10us later
        for dt_i in range(NDT):
            q = dma_q[dt_i % 2]
            q.dma_start(out=wv_sb[:, dt_i], in_=wqkv.ap()[:, 2, dt_i])
            for cq in range(2):
                q.dma_start(
                    out=xt_sb[:, dt_i, cq * 512 : (cq + 1) * 512],
                    in_=xt.ap()[:, dt_i, cq * 512 : (cq + 1) * 512],
                )
        for dt_i in range(NDT):
            q = dma_q[dt_i % 2]
            for cq in range(2, 4):
                q.dma_start(
                    out=xt_sb[:, dt_i, cq * 512 : (cq + 1) * 512],
                    in_=xt.ap()[:, dt_i, cq * 512 : (cq + 1) * 512],
                )
        # wk/wq halves ride both queues after the xt streams: landed by ~20us
        # on every core (kT starts ~28us) without delaying the v-proj's data
        nc.scalar.dma_start(out=wk_sb[:, 0:4], in_=wqkv.ap()[:, 1, 0:4])
        nc.sync.dma_start(out=wk_sb[:, 4:8], in_=wqkv.ap()[:, 1, 4:8])
        nc.scalar.dma_start(out=wq_sb[:, 0:4], in_=wqkv.ap()[:, 0, 0:4])
        nc.sync.dma_start(out=wq_sb[:, 4:8], in_=wqkv.ap()[:, 0, 4:8])
        xt_tiles = [xt_sb[:, dt_i] for dt_i in range(NDT)]
        wv_tiles = [wv_sb[:, dt_i] for dt_i in range(NDT)]
        wk_tiles = [wk_sb[:, dt_i] for dt_i in range(NDT)]
        wq_tiles = [wq_sb[:, dt_i] for dt_i in range(NDT)]

        # v_aug: per head-slot sg, 65 cols = [v_sg (64) | ones (1)]
        v_tiles = []
        for nt in range(NNT):
            dst = p_v.tile([P, 8 * 65], BF16, tag=f"v{nt}")
            ones_dst = dst.rearrange("p (h c) -> p h c", c=65)[:, :, 64:65]
            nc.vector.memset(ones_dst, 1.0)
            v_tiles.append(dst)
        # dt-outer over 8 parallel PSUM accumulators (all 8 banks are free at
        # startup): each dt's matmuls issue as soon as that dt slice arrives
        vacc = []
        for nm in ("vA", "vB", "vD"):
            t = ps_mm.tile([P, 1024], F32, tag="mm", name=nm)
            vacc += [t[:, 0:512], t[:, 512:1024]]
        for s in range(2):
            t = ps_av.tile([P, 512], F32, tag="av", name=f"vC{s}")
            vacc.append(t)
        def stage_v(half, k):
            dst = v_tiles[half * 8 + k]
            v_dst = dst.rearrange("p (h c) -> p h c", c=65)[:, :, 0:DH]
            stage_copy(v_dst, vacc[k].rearrange("p (h c) -> p h c", c=DH))

        # first pass dt-outer: each dt's matmuls issue as soon as that dt
        # slice lands, streaming behind the input DMA
        for dt_i in range(NDT):
            for k in range(8):
                nc.tensor.matmul(
                    vacc[k],
                    xt_tiles[dt_i][:, k * P : (k + 1) * P],
                    wv_tiles[dt_i],
                    start=(dt_i == 0),
                    stop=(dt_i == NDT - 1),
                )
        for k in range(8):
            stage_v(0, k)
        # ---- phase B emitters (kT/qT c-tiles) ----
        kT_tiles = []
        qT_tiles = []

        def emit_qk_quarter(which, ct, nch, dst):
            w_tiles = wq_tiles if which == "q" else wk_tiles
            woff = ct * P
            acc = ps_mm.tile([P, 512], F32, tag="mm", name="acc")
            for dt_i in range(NDT):
                nc.tensor.matmul(
                    acc,
                    w_tiles[dt_i][:, woff : woff + P],
                    xt_tiles[dt_i][:, nch * 512 : (nch + 1) * 512],
                    start=(dt_i == 0),
                    stop=(dt_i == NDT - 1),
                )
            stage_copy(dst[:, nch * 512 : (nch + 1) * 512], acc)

        def alloc_qk(which, ct):
            dst = p_qk.tile([P, NSEQ], BF16, tag=f"{which}T{ct}", name=f"{which}T{ct}")
            (kT_tiles if which == "k" else qT_tiles).append(dst)
            return dst

        def emit_qk_tile(which, ct):
            dst = alloc_qk(which, ct)
            for nch in range(NNC):
                emit_qk_quarter(which, ct, nch, dst)

        # second pass k-outer: data is resident by now, so finish group k and
        # stage it while group k+1 still matmuls — the kT phase's PSUM ring
        # slots free up 6 groups earlier (was a ~3.6us PE gap + HAM
        # re-throttle when all 8 copies piled up at the end). kT ct0's four
        # quarters ride between the last v groups: when a v group waits on
        # late xt quarters, the PE chews on kT instead of idling.
        kT0 = alloc_qk("k", 0)
        for k in range(8):
            nt = 8 + k
            for dt_i in range(NDT):
                nc.tensor.matmul(
                    vacc[k],
                    xt_tiles[dt_i][:, nt * P : (nt + 1) * P],
                    wv_tiles[dt_i],
                    start=(dt_i == 0),
                    stop=(dt_i == NDT - 1),
                )
            stage_v(1, k)
            if 3 <= k <= 6:
                emit_qk_quarter("k", 0, k - 3, kT0)
        st_wv.close()

        # ---- phase B: qT ct0 now; the other six qk tiles stream into the
        # first two attention chunks as PAIRS of quarters inserted between a
        # step's av and the next step's dots. The PE queue is in-order, so
        # the inserted quarters execute exactly where the next dots would
        # otherwise sit head-blocked on an exp deadline — the stall time gets
        # filled with useful work. Pairs keep the PSUM ring phase even (an
        # odd insertion would swap the s0/s1 deadline classes). Order
        # guarantees: kT1+qT1 complete inside chunk (0,0) (needed by (0,1)),
        # the rest inside (0,1) (needed by (0,2)+).
        emit_qk_tile("q", 0)
        qk_pend = []
        for ct in range(1, NPAIR):
            for which in ("k", "q"):
                dst = alloc_qk(which, ct)
                qk_pend.extend((which, ct, nch, dst) for nch in range(NNC))

        wout_sb = p_wout.tile([P, NPAIR, D], BF16, tag="wout")
        nc.sync.dma_start(out=wout_sb, in_=wout.ap())
        wout_tiles = [wout_sb[:, ct] for ct in range(NPAIR)]

        # ---- attention: flat software pipeline across chunk boundaries ----
        # Per jp step of the CURRENT chunk: dots -> av of the step one behind
        # (possibly the previous chunk's tail) -> maybe one out-proj group ->
        # exp for this step. The PE's in-order queue therefore never waits on
        # an epilogue chain: epilogues and out-projections overlap the next
        # chunk's dots/av stream (>3.4us PE-idle gaps re-throttle HAM to
        # 1.2GHz, which is what made the naive ordering slow).
        aT_by_ic = {}  # ic -> [aT tile per hp]

        class Cctx:
            def __init__(self, ic, hp):
                self.ic, self.hp, self.i0 = ic, hp, ic * 512
                self.ci = ic * NPAIR + hp
                self.av_ps = [
                    ps_av.tile([65, 512], F32, tag="av", name=f"av{s}")
                    for s in range(2)
                ]
                self.aT = p_aT.tile([P, 512], BF16, tag="aT", name=f"aT{ic}_{hp}")
                aT_by_ic.setdefault(ic, []).append(self.aT)

        def emit_dots(c, jp):
            tiles = []
            for s in range(2):
                tiles.append(ps_mm.tile([P, 1024], F32, tag="mm", name="dots"))
            for half in range(2):
                for s in range(2):
                    r0 = s * DH
                    jtx = 2 * jp + half
                    nc.tensor.matmul(
                        tiles[s][:, half * 512 : (half + 1) * 512],
                        kT_tiles[c.hp][r0 : r0 + DH, jtx * P : (jtx + 1) * P],
                        qT_tiles[c.hp][r0 : r0 + DH, c.i0 : c.i0 + 512],
                        start=True,
                        stop=True,
                        tile_position=(r0, 0),
                    )
            return tiles

        def emit_exp(c, jp, dots_tiles):
            exp_tiles = []
            for s in range(2):
                e = p_exp.tile([P, 1024], BF16, tag="exp")
                if (jp, s) in DVE_EXP:
                    nc.vector.tensor_scalar(
                        out=e.bitcast(I16),
                        in0=dots_tiles[s],
                        scalar1=A_SCH,
                        scalar2=B_SCH,
                        op0=ALU.mult,
                        op1=ALU.add,
                    )
                else:
                    nc.scalar.activation(
                        out=e, in_=dots_tiles[s], func=AF.Exp, scale=EXP_SCALE
                    )
                exp_tiles.append(e)
            return exp_tiles

        def emit_av(c, jp, exp_pair):
            for s in range(2):
                sg = c.hp * 2 + s
                for half in range(2):
                    jtx = 2 * jp + half
                    nc.tensor.matmul(
                        c.av_ps[s],
                        v_tiles[jtx][:, sg * 65 : sg * 65 + 65],
                        exp_pair[s][:, half * 512 : (half + 1) * 512],
                        start=(jp == 0 and half == 0),
                        stop=(jp == NJP - 1 and half == 1),
                    )

        # Epilogue pipeline. GpSimd is DMA-free after the startup loads and
        # cannot touch PSUM, so its ext-isa ucode stays resident and it does
        # the SBUF-only normalize divides; ScalarE evacuates av PSUM; the
        # denominator row is replicated across partitions by a sync-queue DMA
        # reading the av_sb row with a stride-0 partition broadcast. No
        # reciprocal, no casting DMA, no PE broadcast matmul.
        def emit_epilogue_a(c, e):
            # evacuate av PSUM into one merged SBUF tile (one copy per
            # engine, in parallel); hop the denominator row out as [8,128]
            # so the reciprocal runs on eight partitions at 1/8 the cost.
            # (Quarter-width copies split across both engines were tried to
            # cut the bank-release latency — 80us SLOWER, don't.)
            av_sb = p_avsb.tile([65, 1024], F32, tag="av_sb", name="avsb")
            nc.vector.tensor_copy(av_sb[:, 0:512], c.av_ps[0])
            nc.scalar.copy(av_sb[:, 512:1024], c.av_ps[1])
            e["av_sb"] = av_sb
            den_sb = p_recip.tile([8, 128], F32, tag="den_sb")
            nc.sync.dma_start(out=den_sb, in_=av_sb[64:65, :])
            e["den_sb"] = den_sb

        def emit_epilogue_b1(c, e):
            # fast reciprocal on DVE, then bounce the rows to DRAM (sync);
            # DRAM layout comes out linear: recip_s0[512] | recip_s1[512]
            recip = p_recip.tile([8, 128], F32, tag="recip")
            nc.vector.reciprocal_approx_fast(out=recip, in_=e["den_sb"])
            e["scratch"] = rscratch.ap()[c.ci % 3 : c.ci % 3 + 1]
            nc.sync.dma_start(out=e["scratch"], in_=recip)

        def emit_epilogue_b2(c, e):
            # DMA the recip row back with a stride-0 outer dim: a 64-way
            # partition replicate with no engine work at all
            bc = p_bcast.tile([DH, 1024], F32, tag="bc", name="bc")
            nc.sync.dma_start(out=bc, in_=e["scratch"].partition_broadcast(DH))
            e["bc"] = bc

        def emit_epilogue_b3(c, e):
            # normalize on GpSimd (bf16 out); s=1 reaches aT partitions
            # 64:127 via an SBUF-to-SBUF DMA hop (engines can't cross
            # partitions)
            av_sb, bc = e["av_sb"], e["bc"]
            nc.gpsimd.tensor_mul(c.aT[0:DH, :], av_sb[0:DH, 0:512], bc[:, 0:512])
            tmp = p_tmp.tile([DH, 512], BF16, tag="tmp")
            nc.gpsimd.tensor_mul(tmp, av_sb[0:DH, 512:1024], bc[:, 512:1024])
            nc.sync.dma_start(out=c.aT[DH:P, :], in_=tmp)

        from collections import deque

        pend_av = deque()  # (cctx, jp, exp_tiles); av lags dots by AV_LAG steps
        AV_LAG = 5  # exp gets five pipeline steps of slack before av needs it
        pend_tasks = []  # [countdown, fn] epilogue stages

        def pump(drain=False):
            if pend_av and (len(pend_av) >= AV_LAG or drain):
                pc, pjp, pexp = pend_av.popleft()
                emit_av(pc, pjp, pexp)
                if pjp == NJP - 1:
                    e = {}
                    emit_epilogue_a(pc, e)
                    for delay, fn in (
                        (2, emit_epilogue_b1),
                        (4, emit_epilogue_b2),
                        (6, emit_epilogue_b3),
                    ):
                        pend_tasks.append([delay, fn, pc, e])
            for t in list(pend_tasks):
                t[0] -= 1
                if t[0] <= 0:
                    t[1](t[2], t[3])
                    pend_tasks.remove(t)

        chunks = [(ic, hp) for ic in range(NNC) for hp in range(NPAIR)]
        for ci, (ic, hp) in enumerate(chunks):
            c = Cctx(ic, hp)
            # safety flush: any qk quarters not yet streamed must land before
            # a chunk that reads their tiles (pacing drains them by ci==2)
            while ci >= 2 and qk_pend:
                for _ in range(2):
                    if qk_pend:
                        emit_qk_quarter(*qk_pend.pop(0))
            for jp in range(NJP):
                dots_tiles = emit_dots(c, jp)
                pump()
                if ci < 2 and jp >= 2 and qk_pend:
                    emit_qk_quarter(*qk_pend.pop(0))
                    emit_qk_quarter(*qk_pend.pop(0))
                exp_tiles = emit_exp(c, jp, dots_tiles)
                pend_av.append((c, jp, exp_tiles))
        # drain av tail + epilogues
        while pend_av or pend_tasks:
            pump(drain=True)

        # ---- out-projection phase: all 16 n-tiles after attention ----
        # Runs as a dense PE-bound burst (64 matmuls of N=1024, ~27us).
        # Keeping it out of the attention loop frees 2 PSUM banks, which the
        # dots pool uses for 3-deep buffering — that breaks the
        # dots->exp->dots PSUM-reuse serialization that dominated the
        # attention steady state. nt order puts ic=3 last, so the final
        # chunk's epilogue chain hides behind ~20us of earlier out-proj work.
        for nt in range(NNT):
            ic, lnt = nt // 4, nt % 4
            o_ps = ps_mm.tile([P, D], F32, tag="mm", name=f"o{nt}")
            for ec in range(2):
                for hp in range(NPAIR):
                    nc.tensor.matmul(
                        o_ps[:, ec * 512 : (ec + 1) * 512],
                        aT_by_ic[ic][hp][:, lnt * P : (lnt + 1) * P],
                        wout_tiles[hp][:, ec * 512 : (ec + 1) * 512],
                        start=(hp == 0),
                        stop=(hp == NPAIR - 1),
                    )
            o_sb = p_ostage.tile([P, D], BF16, tag="o_sb", name=f"o_sb{nt}")
            stage_copy(o_sb, o_ps)
            nc.sync.dma_start(out=out_t[nt], in_=o_sb)

        st_wk.close()
        st_xt.close()

    nc.compile()
    return nc


_NC = None


def _get_program():
    global _NC
    if _NC is None:
        _NC = build_program()
    return _NC


INNER = 1024
BF = ml_dtypes.bfloat16


def kernel(x, W_qkv, W_out, b_out):
    x = np.asarray(x, dtype=np.float32)
    W_qkv = np.asarray(W_qkv, dtype=np.float32)
    W_out = np.asarray(W_out, dtype=np.float32)
    b_out = np.asarray(b_out, dtype=np.float32)
    B = x.shape[0]

    nc = _get_program()
    in_maps = []
    for b in range(B):
        # partition-major shuffles so per-partition DMA runs are contiguous
        xt_sh = np.ascontiguousarray(
            x[b].T.reshape(NDT, P, NSEQ).transpose(1, 0, 2)
        ).astype(BF)
        for hh in range(2):
            cs = hh * CH
            wq = W_qkv[:, cs : cs + CH]
            wk = W_qkv[:, INNER + cs : INNER + cs + CH]
            wv = W_qkv[:, 2 * INNER + cs : 2 * INNER + cs + CH]
            wqkv_sh = np.ascontiguousarray(
                np.stack([wq, wk, wv])  # [3, 1024, 512]
                .reshape(3, NDT, P, CH)
                .transpose(2, 0, 1, 3)
            ).astype(BF)
            wout_sh = np.ascontiguousarray(
                W_out[cs : cs + CH, :].reshape(NPAIR, P, D).transpose(1, 0, 2)
            ).astype(BF)
            in_maps.append(
                {
                    "xt": xt_sh,
                    "wqkv": wqkv_sh,
                    "wout": wout_sh,
                    "ones": np.ones((P, 1), dtype=BF),
                }
            )
    res = run_bass_kernel_spmd(nc, in_maps, core_ids=list(range(8)))
    out = np.empty((B, NSEQ, D), dtype=np.float32)
    for b in range(B):
        out[b] = (
            res.results[2 * b]["out"].astype(np.float32)
            + res.results[2 * b + 1]["out"].astype(np.float32)
            + b_out
        )
    return out

